# revision 45
# baseline (speedup 1.0000x reference)
"""Trainium2 Bass kernel for nn_Axial_PFCU_Continuous (dense_cnn).

Math (per sample, C=96, H=W=128), folded host-side:
  m+l   = cA0 (.) x + sum_d chA[d] (.) shiftH(x,d) + sum_d cwA[d] (.) shiftW(x,d)
  z     = Wf~ @ (m+l) + anchor;  anchor = cB0 (.) x + 4 edge taps + bias
  pre   = PReLU(z, a)
  coord attention: spatial means of pre -> tiny matmuls -> sigmoid gates
  out   = pre * ah(c,h) * aw(c,w)

Sharding: pure data-parallel, 1 of 8 batch samples per NeuronCore.

Per core (C=96 partitions, H*W free), per 4-row psum chunk (512 f32 cols):
  PE  : 8 fp8e4 DoubleRow PAIR matmuls covering all 12 shift terms of m+l
        and the 4 dwconv edge taps (each pair = 2 accumulation terms at 0.5
        cycles/row), reading a zero-padded per-channel-scaled fp8 image so
        shifts are pure AP offsets with no edge masking, plus one bf16
        matmul W'@x (W' = wfuse~.diag(cA0)+diag(cB0) carries the dominant
        identity terms at bf16 precision). ~1067ns/chunk -> PE ~34us/core.
  ACT : merged 8-row PReLU psum evict (BN bias folded into the activation),
        per-block coord-att PReLU/Sigmoid, half the output DMAs
  DVE : xw column sums (fp16), xh fold+reduce, paired final x aw gates
  GPS : in-place x ah gate per block (hidden under the PE phase), late-unit
        xh folds, part of the x8/output DMAs

fp8 precision recipe: stationary rows and the fp8 x copy carry a per-channel
scale s[c] = 8*u[c] (u grid-tuned so the edge-tap diagonals quantize well);
identity/BN terms never leave bf16. Measured rel err ~1.45e-2 vs the f32
reference (gate 2e-2).

Schedule: PE warm-up from a memset tile at ~0.6us; x + x8 pieces interleaved
across the SP/ACT/gpsimd DMA queues during fill; the last block is split in
two 4-row units and its column sums are folded off the xw4 chain so the aw
sigmoid lands ~2us after the last matmul; finals (x aw + output DMA) are
spread over DVE/GPS and all three DMA queues.
"""
import sys
import math

sys.path.insert(0, '/opt/trn_rl_repo')

import numpy as np
import ml_dtypes
from contextlib import ExitStack

import concourse.bass as bass
import concourse.bacc as bacc
from concourse import mybir, tile
from concourse.bass_utils import run_bass_kernel_spmd

f32 = mybir.dt.float32
bf16 = mybir.dt.bfloat16
fp16 = mybir.dt.float16
f8 = mybir.dt.float8e4
ALU = mybir.AluOpType
AF = mybir.ActivationFunctionType
PM = mybir.MatmulPerfMode

B, C, H, W = 8, 96, 128, 128
HW = H * W
EPS = 1e-5
N_CORES = 8
MIP = 8

NBLK = 16           # h-blocks per sample; one 8-row psum chunk per block
BH = H // NBLK

_GRAPH_CACHE = {}


# ----------------------------------------------------------------- host folds
def _taps(w_taps, r):
    """offset -> (C,) coefficient for the integer-shift decomposition."""
    r = max(float(r), 1.0)
    K = w_taps.shape[1]
    d2w = {}
    for i in range(K):
        s = (i - K // 2) * r
        f = math.floor(s)
        frac = s - f
        for d, wt in ((int(f), 1.0 - frac), (int(f) + 1, frac)):
            if wt != 0.0:
                if d not in d2w:
                    d2w[d] = np.zeros(C, np.float64)
                d2w[d] = d2w[d] + wt * np.asarray(w_taps[:, i], np.float64)
    return {d: w for d, w in d2w.items() if abs(d) < H}


def _merge(a, b):
    out = dict(a)
    for d, w in b.items():
        out[d] = out.get(d, np.zeros(C, np.float64)) + w
    return out


def _pairs(offsets):
    """Group offsets into pairs (symmetric +-d together when possible).
    Odd leftover is paired with a zero-coefficient duplicate marker None."""
    offs = sorted(offsets, key=lambda d: (abs(d), d))
    out = []
    used = set()
    for d in offs:
        if d in used:
            continue
        if -d in offs and -d not in used and d != -d:
            out.append((d, -d) if d < 0 else (-d, d))
            used.add(d); used.add(-d)
    rest = [d for d in offs if d not in used]
    for i in range(0, len(rest) - 1, 2):
        out.append((rest[i], rest[i + 1]))
    if len(rest) % 2:
        out.append((rest[-1], None))
    return out


class _Pack:
    def __init__(self, rows=C):
        self.cols = {}
        self.parts = []
        self.pos = 0
        self.rows = rows

    def put(self, name, arr):
        arr = np.asarray(arr, np.float64)
        if arr.ndim == 1:
            arr = arr[:, None]
        pad = np.zeros((self.rows, arr.shape[1]), np.float64)
        pad[:arr.shape[0], :] = arr
        self.cols[name] = (self.pos, arr.shape[1])
        self.parts.append(pad)
        self.pos += arr.shape[1]

    def done(self, dt, min_cols=0):
        if self.pos < min_cols:
            self.put('_pad', np.zeros((self.rows, min_cols - self.pos)))
        return np.concatenate(self.parts, axis=1).astype(dt)


def _fold(inp):
    g = lambda k: np.asarray(inp[k], np.float64)
    hA = _merge(_taps(g('wh_m'), float(np.asarray(inp['r_m']))),
                _taps(g('wh_l'), float(np.asarray(inp['r_l']))))
    wA = _merge(_taps(g('ww_m'), float(np.asarray(inp['r_m']))),
                _taps(g('ww_l'), float(np.asarray(inp['r_l']))))
    hA[0] = hA.get(0, np.zeros(C)) + 2.0    # identity terms of m+l
    wA.setdefault(0, np.zeros(C))

    h_offs = tuple(d for d in sorted(hA) if d != 0)
    w_offs = tuple(d for d in sorted(wA) if d != 0)
    h_pairs = _pairs(h_offs)
    w_pairs = _pairs(w_offs)
    halo_h = max([1] + [abs(d) for d in h_offs])
    halo_w = max([1] + [abs(d) for d in w_offs])
    PH, PW = H + 2 * halo_h, W + 2 * halo_w

    sf = g('bnf_g') / np.sqrt(g('bnf_v') + EPS)
    wfuse_t = (g('w_fuse') * sf[:, None]).T.copy()      # (Cin, Cout) lhsT
    bf = g('bnf_b') - g('bnf_m') * sf

    ds = g('dg_g') / np.sqrt(g('dg_v') + EPS)
    db = g('dg_b') - g('dg_m') * ds
    dg_wh, dg_ww = g('dg_wh'), g('dg_ww')
    ehm1, eh0, ehp1 = ds * dg_wh[:, 0], ds * (dg_wh[:, 1] + 1.0), ds * dg_wh[:, 2]
    ewm1, ew0, ewp1 = ds * dg_ww[:, 0], ds * dg_ww[:, 1], ds * dg_ww[:, 2]

    cA0 = hA[0] + wA[0]
    cB0 = eh0 + ew0
    bz = bf + db

    cs = g('ca_g') / np.sqrt(g('ca_v') + EPS)
    cb = g('ca_b') - g('ca_m') * cs

    # f32 consts (biases, PReLU params, CA chain scalars)
    pkf = _Pack()
    pkf.put('bz', bz)
    pkf.put('act_a', g('act_a'))
    pkf.put('zero', np.zeros(C))
    pkf.put('cas', cs); pkf.put('cab', cb); pkf.put('caa', g('ca_a'))
    consts = pkf.done(np.float32)

    # bf16 consts (main stationary + CA stationaries); padded for PE warmers
    Wp = wfuse_t * cA0[:, None] + np.diag(cB0)
    pkb = _Pack()
    pkb.put('Wp', Wp)
    pkb.put('caw1_t', (g('ca_w1') / float(W)).T)   # (C, 8); 1/W mean fold
    pkb.put('cawh_t', g('ca_wh').T)                # (8, C)
    pkb.put('caww_t', g('ca_ww').T)
    constb = pkb.done(ml_dtypes.bfloat16, min_cols=512)

    # per-channel fp8 scale s[c] = 8*u[c]: the 8x lifts the folded wfuse*chA
    # stationaries out of subnormal range; u is grid-tuned so the four edge
    # taps quantize well (the A rows are insensitive to the exact scale).
    # x8 carries x/s; stationary rows carry *s, so products are exact in s.
    f8r = lambda a: np.asarray(a, np.float32).astype(
        ml_dtypes.float8_e4m3).astype(np.float64)
    taps4 = np.stack([ehm1, ehp1, ewm1, ewp1], 0)
    u = np.ones(C)
    cands = np.linspace(0.6, 1.45, 160)
    for c in range(C):
        errs = [np.sum((f8r(taps4[:, c] * 8 * uu) / (8 * uu)
                        - taps4[:, c]) ** 2) for uu in cands]
        u[c] = cands[int(np.argmin(errs))]
    s8 = 8.0 * u

    # fp8 pair stationaries: [lhsT(d1) | lhsT(d2)] per pair, 192 cols each
    def pairblk(cmap, pairs, diag):
        blocks = []
        for d1, d2 in pairs:
            for d in (d1, d2):
                coef = cmap[d] if d is not None else np.zeros(C)
                blocks.append(np.diag(coef * s8) if diag else
                              wfuse_t * (np.asarray(coef) * s8)[:, None])
        return blocks

    pk8 = _Pack()
    for i, blkpair in enumerate(_chunks(pairblk(hA, h_pairs, False), 2)):
        pk8.put(f'AH{i}', np.concatenate(blkpair, axis=1))
    for i, blkpair in enumerate(_chunks(pairblk(wA, w_pairs, False), 2)):
        pk8.put(f'AW{i}', np.concatenate(blkpair, axis=1))
    pk8.put('BH0', np.concatenate(
        [np.diag(ehm1 * s8), np.diag(ehp1 * s8)], axis=1))
    pk8.put('BW0', np.concatenate(
        [np.diag(ewm1 * s8), np.diag(ewp1 * s8)], axis=1))
    constf8 = pk8.done(ml_dtypes.float8_e4m3)

    bh_pairs = [(-1, 1)]
    bw_pairs = [(-1, 1)]
    key = (tuple(h_pairs), tuple(w_pairs), PH, PW,
           consts.shape[1], constb.shape[1], constf8.shape[1])
    meta = dict(h_pairs=h_pairs, w_pairs=w_pairs,
                bh_pairs=bh_pairs, bw_pairs=bw_pairs,
                PH=PH, PW=PW, halo_h=halo_h, halo_w=halo_w,
                colf=pkf.cols, colb=pkb.cols, col8=pk8.cols,
                ckf=consts.shape[1], ckb=constb.shape[1],
                ck8=constf8.shape[1], s8=s8)
    return consts, constb, constf8, meta, key


def _chunks(lst, n):
    return [lst[i:i + n] for i in range(0, len(lst), n)]


# -------------------------------------------------------------- graph builder
def _build(meta):
    h_pairs, w_pairs = meta['h_pairs'], meta['w_pairs']
    bh_pairs, bw_pairs = meta['bh_pairs'], meta['bw_pairs']
    PH, PW = meta['PH'], meta['PW']
    halo_h, halo_w = meta['halo_h'], meta['halo_w']
    colf, colb, col8 = meta['colf'], meta['colb'], meta['col8']

    nc = bacc.Bacc()
    x_p = nc.declare_dram_parameter("x", (C, HW), bf16, isOutput=False)
    x8_p = nc.declare_dram_parameter("x8", (C, PH * PW), f8, isOutput=False)
    cf_p = nc.declare_dram_parameter("consts", (C, meta['ckf']), f32,
                                     isOutput=False)
    cb_p = nc.declare_dram_parameter("constb", (C, meta['ckb']), bf16,
                                     isOutput=False)
    c8_p = nc.declare_dram_parameter("constf8", (C, meta['ck8']), f8,
                                     isOutput=False)
    o_p = nc.declare_dram_parameter("out", (C, HW), bf16, isOutput=True)

    with tile.TileContext(nc) as tc, ExitStack() as ctx:
        big = ctx.enter_context(tc.tile_pool(name="big", bufs=1))
        sm = ctx.enter_context(tc.tile_pool(name="sm", bufs=4))
        psq = ctx.enter_context(tc.tile_pool(name="psq", bufs=3, space="PSUM"))
        pss = ctx.enter_context(tc.tile_pool(name="pss", bufs=1, space="PSUM"))
        paw = ctx.enter_context(tc.tile_pool(name="paw", bufs=1, space="PSUM"))

        cst = big.tile([C, meta['ckf']], f32, tag="cst")
        cbt = big.tile([C, meta['ckb']], bf16, tag="cbt")
        c8t = big.tile([C, meta['ck8']], f8, tag="c8t")
        x_sb = big.tile([C, HW], bf16, tag="x")
        x8_sb = big.tile([C, PH * PW], f8, tag="x8")

        # --- DMA schedule -------------------------------------------------
        # x8 pieces: padded-row groups, contiguous in DRAM and SBUF.
        rpp = max(1, 1600 // PW)             # ~1.6KB pieces
        np8 = (PH + rpp - 1) // rpp
        p8 = []
        for p in range(np8):
            a = p * rpp * PW
            b = min((p + 1) * rpp, PH) * PW
            p8.append((a, b))
        # consts on the ACT queue (idle early; warmers only need cbt which
        # lands first); x chunks + early x8 pieces interleave on SP; the
        # first halo pieces and the late pieces go via the gpsimd SWDGE.
        nc.scalar.dma_start(c8t[:], c8_p[:])
        nc.scalar.dma_start(cbt[:], cb_p[:])
        nc.scalar.dma_start(cst[:], cf_p[:])
        sp_seq = [('p8', 0), ('p8', 1), ('x', 0), ('x', 1), ('p8', 4),
                  ('x', 2), ('p8', 5), ('x', 3), ('p8', 6), ('x', 4),
                  ('p8', 7), ('x', 5)] + [('x', j) for j in range(6, NBLK)]
        for kind, idx in sp_seq:
            if kind == 'x':
                sl = slice(idx * HW // NBLK, (idx + 1) * HW // NBLK)
                nc.sync.dma_start(x_sb[:, sl], x_p[:, sl])
            elif idx < np8:
                a, b = p8[idx]
                nc.sync.dma_start(x8_sb[:, a:b], x8_p[:, a:b])
        for p in list(range(2, 4)) + list(range(8, np8)):
            a, b = p8[p]
            nc.gpsimd.dma_start(x8_sb[:, a:b], x8_p[:, a:b])

        def cc(name, rows=C):
            p0, n = colf[name]
            return cst[0:rows, p0:p0 + 1]

        def cb_(name, rows=C):
            p0, n = colb[name]
            return cbt[0:rows, p0:p0 + n]

        def c8_(name):
            p0, n = col8[name]
            return c8t[:, p0:p0 + n].rearrange("p (two m) -> p two m", two=2)

        zcol = cc('zero')

        # --- warmup: ACT tables once; PE p-state ramp from ~0.6us on a
        # memset tile (no DMA dependency) so real blocks start at full clock
        wrm = sm.tile([C, 4], f32, tag="wrm")
        nc.scalar.activation(wrm[:, 0:1], zcol, AF.Prelu, bias=zcol,
                             scale=1.0, alpha=cc('act_a'))
        nc.scalar.activation(wrm[:, 1:2], zcol, AF.Sigmoid, bias=zcol,
                             scale=1.0)
        wmt = sm.tile([C, 512], bf16, tag="wmt")
        nc.vector.memset(wmt[:], 0)
        psw = pss.tile([C, 512], f32, tag="small")
        for wi in range(5):
            nc.tensor.matmul(psw[:], wmt[0:C, 0:C], wmt[0:C, 0:512],
                             start=(wi == 0), stop=(wi == 4))

        ac_sb = big.tile([C, HW], bf16, tag="ac")
        ac3 = ac_sb[:].rearrange("p (h w) -> p h w", w=W)
        x8b = x8_sb[:]

        xw4 = big.tile([C, 4 * W], fp16, tag="xw4")
        nc.vector.memset(xw4[:], 0)
        yin = big.tile([C, H], bf16, tag="yin")
        yinw = big.tile([C, W], bf16, tag="yinw")
        ah = big.tile([C, H], bf16, tag="ah")

        CH = 4              # psum chunk rows (512 f32 cols = one PSUM bank)

        def pair_rhs(r0, pair, axis):
            """Manual AP: (part, 2, CH rows, W cols) windows of padded x8."""
            d1, d2 = pair
            if d2 is None:
                d2 = d1
            if axis == 'h':
                o1 = (halo_h + r0 + d1) * PW + halo_w
                o2 = (halo_h + r0 + d2) * PW + halo_w
            else:
                o1 = (halo_h + r0) * PW + halo_w + d1
                o2 = (halo_h + r0) * PW + halo_w + d2
            ap_list = [list(x8b.ap[0]), [int(o2 - o1), 2], [PW, CH], [1, W]]
            return bass.AP(tensor=x8b.tensor, offset=x8b.offset + o1,
                           ap=ap_list)

        mms = ([(f'AH{i}', p, 'h') for i, p in enumerate(h_pairs)]
               + [(f'AW{i}', p, 'w') for i, p in enumerate(w_pairs)]
               + [('BH0', bh_pairs[0], 'h'), ('BW0', bw_pairs[0], 'w')])

        # --- main pipeline: 8-row blocks (two 4-row psum groups sharing a
        # 2-bank psum tile, single merged evict); the last block is split
        # into two 4-row units so the aw-chain dependency drains faster.
        # The per-block coord-att (ah) chain is emitted one block late so
        # the tiny PE matmuls never stall the in-order PE queue.
        bounds = ([(j * BH, (j + 1) * BH) for j in range(NBLK - 1)]
                  + [(H - BH, H - CH), (H - CH, H)])

        def ah_chain(r0, r1):
            bh = r1 - r0
            pblk = pss.tile([C, 16], f32, tag="small")
            y1b = pblk[0:MIP, 0:bh]
            nc.tensor.matmul(y1b, cb_('caw1_t'), yin[:, r0:r1],
                             start=True, stop=True)
            y2b = sm.tile([MIP, BH], bf16, tag="y2b")
            nc.scalar.activation(y2b[:, 0:bh], y1b, AF.Prelu,
                                 bias=cc('cab', rows=MIP),
                                 scale=cc('cas', rows=MIP),
                                 alpha=cc('caa', rows=MIP))
            ahp = pblk[:, 8:8 + bh]
            nc.tensor.matmul(ahp, cb_('cawh_t', rows=MIP), y2b[:, 0:bh],
                             start=True, stop=True)
            nc.scalar.activation(ah[:, r0:r1], ahp, AF.Sigmoid,
                                 bias=zcol, scale=1.0)
            # apply ah in place on the idle gpsimd engine
            ah_b = ah[:, r0:r1].unsqueeze(2).broadcast_to((C, bh, W))
            nc.gpsimd.tensor_tensor(ac3[:, r0:r1, :],
                                    ac3[:, r0:r1, :], ah_b, op=ALU.mult)

        late_reds = []

        def xh_block(r0, r1, late=False):
            bh = r1 - r0
            fold = sm.tile([C, BH * (W // 2)], bf16, tag="fold")
            f3 = fold[:].rearrange("p (h w) -> p h w", w=W // 2)[:, 0:bh, :]
            eng = nc.gpsimd if late else nc.vector
            eng.tensor_tensor(f3, ac3[:, r0:r1, 0:W // 2],
                              ac3[:, r0:r1, W // 2:W], op=ALU.add)
            red = f3
            if late:
                # second fold on gpsimd; only a tiny reduce rides the DVE
                # queue (emitted after the aw-critical xwA folds)
                f4 = f3[:, :, 0:W // 4]
                nc.gpsimd.tensor_tensor(f4, f3[:, :, 0:W // 4],
                                        f3[:, :, W // 4:W // 2], op=ALU.add)
                late_reds.append((r0, r1, f4))
                return
            with nc.allow_low_precision(reason="xh sums feed smooth gates"):
                nc.vector.tensor_reduce(yin[:, r0:r1], red,
                                        axis=mybir.AxisListType.X, op=ALU.add)

        # the last units' xh chains run on gpsimd so the in-order DVE queue
        # unlocks the aw chain (and with it the finals phase) immediately
        # after the last xw add
        LATE = 3
        prev = None
        for r0, r1 in bounds:
            bh = r1 - r0
            sl = slice(r0 * W, r1 * W)
            pk = psq.tile([C, bh, W], f32, tag="pk")
            for k in range(bh // CH):
                cr0 = r0 + k * CH
                pkk = pk[:, k * CH:(k + 1) * CH, :]
                for mi, (nm, pair, axis) in enumerate(mms):
                    nc.tensor.matmul(pkk, c8_(nm),
                                     pair_rhs(cr0, pair, axis),
                                     start=(mi == 0), stop=False,
                                     perf_mode=PM.DoubleRow)
                nc.tensor.matmul(pkk, cb_('Wp'),
                                 x_sb[:, cr0 * W:(cr0 + CH) * W],
                                 start=False, stop=True)
            # evict with bias + PReLU (whole block, one ACT op)
            nc.scalar.activation(ac_sb[:, sl], pk[:], AF.Prelu,
                                 bias=cc('bz'), scale=1.0, alpha=cc('act_a'))
            # xw column sums (fp16 accumulator, mod-4 rows); the very last
            # unit skips the accumulator — its rows are folded directly at
            # the tail, keeping the aw-critical chain short
            if (r0, r1) != bounds[-1]:
                for k in range(bh // CH):
                    cs_ = (r0 + k * CH) * W
                    nc.vector.tensor_tensor(xw4[:], xw4[:],
                                            ac_sb[:, cs_:cs_ + CH * W],
                                            op=ALU.add)
            if (r0, r1) not in bounds[-LATE:]:
                # xh row sums: fold halves then reduce
                xh_block(r0, r1)
                if prev is not None:
                    ah_chain(*prev)
                prev = (r0, r1)
            else:
                if prev is not None:
                    ah_chain(*prev)
                    prev = None
                xh_block(r0, r1, late=True)

        # --- tail: aw gate (xw reduced via two folds, 2x DVE mode) --------
        r0l, r1l = bounds[-1]
        xwA = sm.tile([C, 2 * W], fp16, tag="xwA")
        nc.vector.tensor_tensor(xwA[:], xw4[:, 0:2 * W], xw4[:, 2 * W:4 * W],
                                op=ALU.add)
        nc.vector.tensor_tensor(xwA[:, 0:W], xwA[:, 0:W], xwA[:, W:2 * W],
                                op=ALU.add)
        # last unit's rows folded straight from ac (off the xw4 chain)
        lf = sm.tile([C, 2 * W], fp16, tag="lf")
        l3 = lf[:].rearrange("p (h w) -> p h w", w=W)
        nc.vector.tensor_tensor(l3, ac3[:, r0l:r0l + 2, :],
                                ac3[:, r0l + 2:r1l, :], op=ALU.add)
        nc.vector.tensor_tensor(lf[:, 0:W], lf[:, 0:W], lf[:, W:2 * W],
                                op=ALU.add)
        nc.vector.tensor_tensor(yinw[:], xwA[:, 0:W], lf[:, 0:W],
                                op=ALU.add)
        for r0, r1, f4 in late_reds:
            with nc.allow_low_precision(reason="xh sums feed smooth gates"):
                nc.vector.tensor_reduce(yin[:, r0:r1], f4,
                                        axis=mybir.AxisListType.X, op=ALU.add)
        pawt = paw.tile([C, 2 * W], f32, tag="aw")
        y1w = pawt[0:MIP, 0:W]
        nc.tensor.matmul(y1w, cb_('caw1_t'), yinw[:], start=True, stop=True)
        y2w = sm.tile([MIP, W], bf16, tag="y2w")
        nc.scalar.activation(y2w[:], y1w, AF.Prelu,
                             bias=cc('cab', rows=MIP),
                             scale=cc('cas', rows=MIP),
                             alpha=cc('caa', rows=MIP))
        awp = pawt[:, W:2 * W]
        nc.tensor.matmul(awp, cb_('caww_t', rows=MIP), y2w[:],
                         start=True, stop=True)
        aw = sm.tile([C, W], bf16, tag="aw")
        nc.scalar.activation(aw[:], awp, AF.Sigmoid, bias=zcol, scale=1.0)

        # late units' ah chains + gates, after the aw chain is queued
        # (the two 4-row tail units share one 8-row chain)
        ah_chain(H - 2 * BH, H - BH)
        ah_chain(H - BH, H)

        # finals: DVE takes paired blocks (one op + one DMA per pair), Pool
        # the rest; DMAs alternate the SP/ACT queues, the last one on Pool
        groups = [((0, 1), 'd'), ((2,), 'p'), ((3, 4), 'd'), ((5,), 'p'),
                  ((6, 7), 'd'), ((8,), 'p'), ((9, 10), 'd'), ((11,), 'p'),
                  ((12,), 'p'), ((13,), 'p'), ((14,), 'd'), ((15,), 'd')]
        qs = [nc.sync, nc.scalar]
        for gi, (blks, e) in enumerate(groups):
            r0 = blks[0] * BH
            r1 = (blks[-1] + 1) * BH
            eng = nc.gpsimd if e == 'p' else nc.vector
            awb = aw[:].unsqueeze(1).broadcast_to((C, r1 - r0, W))
            eng.tensor_tensor(ac3[:, r0:r1, :], ac3[:, r0:r1, :],
                              awb, op=ALU.mult)
            for j in blks:
                jr = j * BH
                q = nc.gpsimd if j == 15 else qs[j % 2]
                q.dma_start(o_p[:, jr * W:(jr + BH) * W],
                            ac_sb[:, jr * W:(jr + BH) * W])

    nc.compile()
    return nc


def _get_graph(meta, key):
    if key not in _GRAPH_CACHE:
        _GRAPH_CACHE[key] = _build(meta)
    return _GRAPH_CACHE[key]


# ------------------------------------------------------------------ interface
def _run(inputs, trace=False):
    x = np.ascontiguousarray(np.asarray(inputs['x'], np.float32))
    assert x.shape == (B, C, H, W)
    consts, constb, constf8, meta, key = _fold(inputs)
    nc = _get_graph(meta, key)
    PH, PW = meta['PH'], meta['PW']
    hh, hw = meta['halo_h'], meta['halo_w']
    xb = x.astype(ml_dtypes.bfloat16)
    xs = (x / meta['s8'][None, :, None, None].astype(np.float32))
    xpad = np.zeros((B, C, PH, PW), ml_dtypes.float8_e4m3)
    xpad[:, :, hh:hh + H, hw:hw + W] = xs.astype(ml_dtypes.float8_e4m3)
    in_maps = []
    for i in range(N_CORES):
        in_maps.append({'x': xb[i].reshape(C, HW).copy(),
                        'x8': xpad[i].reshape(C, PH * PW).copy(),
                        'consts': consts, 'constb': constb,
                        'constf8': constf8})
    res = run_bass_kernel_spmd(nc, in_maps, list(range(N_CORES)), trace=trace)
    out = np.stack([res.results[i]['out'].astype(np.float32).reshape(C, H, W)
                    for i in range(N_CORES)], axis=0)
    return out, res


def kernel(**inputs):
    out, _ = _run(inputs, trace=False)
    return out
